# revision 1
# baseline (speedup 1.0000x reference)
"""GroupGRUCell with shared schema-pool parameters — Trainium2 Bass kernel.

Problem shapes (hardcoded): B=256 batch, U=64 GRU units, DIN=H=256, S=8 schemas.
  Wx[u] = sum_s sw_x[u,s] * pool_x[s].T   (per-unit weights from shared pool)
  gate_x = x @ Wx ; gate_h = h @ Wh ; standard GRU cell gate math.

Sharding: unit-parallel across 8 NeuronCores (8 units per core); the schema
pool is replicated per core.

Per core the schema combine runs on the TENSOR engine (not DVE/ACT) via a
block-diagonal stationary: with d = dchunk*16 + dp,
  lhsT[(s,dp), (u,dp')] = sw[u,s] * delta(dp,dp')      (128x128, host-built)
  rhs [(s,dp), (dchunk,o)] = pool[s, o, d]             (pool streamed once)
  out [(u,dp), (dchunk,o)] = W[u, d, o]                (full K=128 utilization)
PSUM chunks are cast to bf16 SBUF (ACT/DVE alternate), then a strided
SBUF->SBUF DMA regroups partitions (u,dp) -> d%128 per unit ("shuffle"),
which restores the natural d-order that the host-side xT layout uses.
Gate matmuls and GRU gate math follow the standard structure; elementwise
work is spread over ACT (sigmoid/tanh), DVE (mults/adds), GPSIMD (sub/add).
"""

import numpy as np
import ml_dtypes

B, U, DIN, H, S = 256, 64, 256, 256, 8
NCORES = 8
UC = U // NCORES  # units per core
O3 = 3 * H        # 768
KC = DIN // 128   # 2 contraction chunks
MC = B // 128     # 2 batch chunks
DC = 16           # dchunk count (d = dchunk*16 + dp)
DP = 16           # dp count

BF16 = ml_dtypes.bfloat16


def _build_program():
    from contextlib import ExitStack

    import concourse.bacc as bacc
    import concourse.mybir as mybir
    import concourse.tile as tile

    bf = mybir.dt.bfloat16
    f32 = mybir.dt.float32
    AF = mybir.ActivationFunctionType
    ALU = mybir.AluOpType

    nc = bacc.Bacc("TRN2", target_bir_lowering=False, debug=False)

    poolx = nc.dram_tensor("poolx", [128, 8, 2 * O3], bf, kind="ExternalInput")
    poolh = nc.dram_tensor("poolh", [128, 8, 2 * O3], bf, kind="ExternalInput")
    swx = nc.dram_tensor("swx", [128, 128], bf, kind="ExternalInput")
    swh = nc.dram_tensor("swh", [128, 128], bf, kind="ExternalInput")
    xt = nc.dram_tensor("xt", [128, UC, KC, B], bf, kind="ExternalInput")
    ht = nc.dram_tensor("ht", [128, UC, KC, B], bf, kind="ExternalInput")
    hbh = nc.dram_tensor("hbh", [128, UC, MC, H], bf, kind="ExternalInput")
    hy = nc.dram_tensor("hy", [128, UC, MC, H], bf, kind="ExternalOutput")

    with tile.TileContext(nc) as tc, ExitStack() as ctx:
        pconst = ctx.enter_context(tc.tile_pool(name="pconst", bufs=1))
        pgtmp = ctx.enter_context(tc.tile_pool(name="pgtmp", bufs=3))

        # --- input loads (priority order on the SP queue) ---
        swx_sb = pconst.tile([128, 128], bf, tag="swx")
        nc.sync.dma_start(out=swx_sb, in_=swx[:, :])
        swh_sb = pconst.tile([128, 128], bf, tag="swh")
        nc.sync.dma_start(out=swh_sb, in_=swh[:, :])
        qeng = (nc.sync, nc.scalar, nc.gpsimd)
        # per-g tiles so each combine matmul / shuffle depends only on its own
        # slice's DMA (tile-granular dependency tracking)
        # pool-x on the two HWDGE queues (kept clear for the shuffles);
        # pool-h + bulk gate inputs on the gpsimd SWDGE queue
        pool_g = {}
        for t, dram in (("x", poolx), ("h", poolh)):
            for g in range(8):
                pool_g[(t, g)] = pconst.tile(
                    [128, 2 * O3], bf, tag=f"pool{t}{g}", name=f"pool{t}{g}"
                )
                eng = qeng[g % 2] if t == "x" else nc.gpsimd
                eng.dma_start(out=pool_g[(t, g)], in_=dram[:, g, :])
        xt_sb = pconst.tile([128, UC, KC, B], bf, tag="xt")
        nc.gpsimd.dma_start(out=xt_sb, in_=xt[:, :, :, :])
        ht_sb = pconst.tile([128, UC, KC, B], bf, tag="ht")
        nc.gpsimd.dma_start(out=ht_sb, in_=ht[:, :, :, :])
        hbh_sb = pconst.tile([128, UC, MC, H], bf, tag="hbh")
        nc.gpsimd.dma_start(out=hbh_sb, in_=hbh[:, :, :, :])

        wsb_g = {
            (t, g): pconst.tile([128, 2 * O3], bf, tag=f"wsb{t}{g}", name=f"wsb{t}{g}")
            for t in ("x", "h")
            for g in range(8)
        }
        # all units' combined weights, gate-matmul layout: [d%128, u, d//128, o]
        wp = {
            "x": pconst.tile([128, UC, KC, O3], bf, tag="wpx", name="wpx"),
            "h": pconst.tile([128, UC, KC, O3], bf, tag="wph", name="wph"),
        }
        out_sb = {
            i: pconst.tile([128, 2, MC, H], bf, tag=f"out{i}", name=f"out{i}")
            for i in range(UC // 2)
        }

        # --- schema combine on the PE + partition-regroup shuffle ---
        # free layout of pool/wsb: (g, (kc, o)) with dchunk = kc*8 + g, so one
        # shuffle DMA per g moves 3072B-contiguous runs per source partition.
        with tc.tile_pool(name="pcomb", bufs=3, space="PSUM") as pcomb:
            cast_rr = 0
            for t, sw_sb in (("x", swx_sb), ("h", swh_sb)):
                for g in range(8):
                    for c in range(3):
                        os = slice(c * 512, (c + 1) * 512)
                        ps = pcomb.tile([128, 512], f32, tag="cps")
                        nc.tensor.matmul(
                            ps, sw_sb, pool_g[(t, g)][:, os],
                            start=True, stop=True,
                        )
                        dst = wsb_g[(t, g)][:, os]
                        if cast_rr % 2 == 0:
                            nc.scalar.activation(out=dst, in_=ps, func=AF.Copy)
                        else:
                            nc.vector.tensor_copy(out=dst, in_=ps)
                        cast_rr += 1
                    # shuffle: wsb partitions are (dp,u) dp-major; src [128,
                    # (kc,o)] iterates ((dp,u), kc, o), matching the legal dst
                    # AP [16 part (dp), u, kc, o] at partition base g*16.
                    qeng[g % 2].dma_start(
                        out=wp[t][g * DP : (g + 1) * DP, :, :, :],
                        in_=wsb_g[(t, g)][:, :],
                    )

        # --- gate matmuls + GRU gate math ---
        with tc.tile_pool(name="pg", bufs=2, space="PSUM") as pg:
            for u in range(UC):
                ri = [
                    pg.tile([128, 512], f32, tag=f"ri{mc}", name=f"ri{mc}")
                    for mc in range(MC)
                ]
                nxh = [
                    pg.tile([128, 512], f32, tag=f"nxh{mc}", name=f"nxh{mc}")
                    for mc in range(MC)
                ]
                for mc in range(MC):
                    bs = slice(mc * 128, (mc + 1) * 128)
                    for t, t_sb, nlo in (("x", xt_sb, 0), ("h", ht_sb, 256)):
                        for kc in range(KC):
                            lhs = t_sb[:, u, kc, bs]
                            nc.tensor.matmul(
                                ri[mc], lhs, wp[t][:, u, kc, 0:512],
                                start=(t == "x" and kc == 0),
                                stop=(t == "h" and kc == 1),
                            )
                            nc.tensor.matmul(
                                nxh[mc][:, nlo : nlo + 256],
                                lhs, wp[t][:, u, kc, 512:O3],
                                start=(kc == 0), stop=(kc == 1),
                            )
                    sig = pgtmp.tile([128, 512], f32, tag="sig")
                    nc.scalar.activation(out=sig, in_=ri[mc], func=AF.Sigmoid)
                    t1 = pgtmp.tile([128, H], f32, tag="t1")
                    nc.vector.tensor_tensor(
                        out=t1, in0=sig[:, 0:H], in1=nxh[mc][:, 256:512], op=ALU.mult
                    )
                    t2 = pgtmp.tile([128, H], f32, tag="t2")
                    nc.vector.tensor_tensor(
                        out=t2, in0=t1, in1=nxh[mc][:, 0:256], op=ALU.add
                    )
                    ng = pgtmp.tile([128, H], f32, tag="ng")
                    nc.scalar.activation(out=ng, in_=t2, func=AF.Tanh)
                    d = pgtmp.tile([128, H], f32, tag="d")
                    nc.vector.tensor_tensor(
                        out=d, in0=hbh_sb[:, u, mc, :], in1=ng, op=ALU.subtract
                    )
                    e = pgtmp.tile([128, H], f32, tag="e")
                    nc.vector.tensor_tensor(
                        out=e, in0=sig[:, H:512], in1=d, op=ALU.mult
                    )
                    nc.vector.tensor_tensor(
                        out=out_sb[u // 2][:, u % 2, mc, :], in0=ng, in1=e, op=ALU.add
                    )
                    if u % 2 == 1:
                        nc.scalar.dma_start(
                            out=hy[:, u - 1 : u + 1, mc, :],
                            in_=out_sb[u // 2][:, :, mc, :],
                        )

    nc.compile()
    return nc


def _prep_inputs(x, hidden, pool_x, pool_h, sw_x, sw_h):
    """Host-side (free) slicing / transposition / casting per core."""
    # pool[s, o, d] -> rhs layout [(s,dp) part, (dchunk, o) free], d = dchunk*16+dp
    def prep_pool(p):
        pt = p.transpose(0, 2, 1)          # [S, DIN, O3]
        pt = pt.reshape(S, KC, 8, DP, O3)  # [s, kc, g, dp, o]  (d = (kc*8+g)*16+dp)
        pt = pt.transpose(0, 3, 2, 1, 4)   # [s, dp, g, kc, o]
        return np.ascontiguousarray(pt.reshape(128, 8, 2 * O3).astype(BF16))

    poolx_h = prep_pool(pool_x)
    poolh_h = prep_pool(pool_h)

    in_maps = []
    for c in range(NCORES):
        us = slice(c * UC, (c + 1) * UC)

        def sw_block(sw_c):  # [UC, S] -> [(s,dp), (dp',u)] block-diagonal
            blk = np.zeros((S, DP, DP, UC), dtype=np.float32)
            for dp in range(DP):
                blk[:, dp, dp, :] = sw_c.T
            return np.ascontiguousarray(blk.reshape(128, 128).astype(BF16))

        xc = x[:, us, :]       # [B, UC, DIN]
        hc = hidden[:, us, :]
        # [128 (d%128), UC, KC (d//128), B]
        xt_h = np.ascontiguousarray(
            xc.transpose(1, 2, 0).reshape(UC, KC, 128, B).transpose(2, 0, 1, 3).astype(BF16)
        )
        ht_h = np.ascontiguousarray(
            hc.transpose(1, 2, 0).reshape(UC, KC, 128, B).transpose(2, 0, 1, 3).astype(BF16)
        )
        # [128 (b%128), UC, MC (b//128), H]
        hbh_h = np.ascontiguousarray(
            hc.reshape(MC, 128, UC, H).transpose(1, 2, 0, 3).astype(BF16)
        )
        in_maps.append(
            {
                "poolx": poolx_h,
                "poolh": poolh_h,
                "swx": sw_block(sw_x[us]),
                "swh": sw_block(sw_h[us]),
                "xt": xt_h,
                "ht": ht_h,
                "hbh": hbh_h,
            }
        )
    return in_maps


_CACHED_NC = None


def _get_nc():
    global _CACHED_NC
    if _CACHED_NC is None:
        _CACHED_NC = _build_program()
    return _CACHED_NC


def kernel(x, hidden, pool_x, pool_h, sw_x, sw_h, _trace=False, _results_holder=None):
    from concourse.bass_utils import run_bass_kernel_spmd

    x = np.asarray(x)
    hidden = np.asarray(hidden)
    pool_x = np.asarray(pool_x)
    pool_h = np.asarray(pool_h)
    sw_x = np.asarray(sw_x)
    sw_h = np.asarray(sw_h)

    nc = _get_nc()
    in_maps = _prep_inputs(x, hidden, pool_x, pool_h, sw_x, sw_h)
    res = run_bass_kernel_spmd(
        nc, in_maps, core_ids=list(range(NCORES)), trace=_trace
    )
    if _results_holder is not None:
        _results_holder.append(res)

    out = np.empty((B, U, H), dtype=np.float32)
    for c in range(NCORES):
        hy_c = np.asarray(res.results[c]["hy"]).astype(np.float32)  # [128, UC, MC, H]
        # out[b, u, h] with b = mc*128 + bp
        out[:, c * UC : (c + 1) * UC, :] = hy_c.transpose(2, 0, 1, 3).reshape(B, UC, H)
    return out



# revision 2
# speedup vs baseline: 1.0614x; 1.0614x over previous
"""GroupGRUCell with shared schema-pool parameters — Trainium2 Bass kernel.

Problem shapes (hardcoded): B=256 batch, U=64 GRU units, DIN=H=256, S=8 schemas.
  Wx[u] = sum_s sw_x[u,s] * pool_x[s].T   (per-unit weights from shared pool)
  gate_x = x @ Wx ; gate_h = h @ Wh ; standard GRU cell gate math.

Sharding: unit-parallel across 8 NeuronCores (8 units per core); the schema
pool is replicated per core.

v2 changes vs the bf16 baseline:
  * pool and the combined weights W are stored as float8 e3m4 scaled by 32
    (W absmax ~0.24 -> ~7.6, well inside e3m4 range; verified rel-err ~1e-2
    vs the 2e-2 gate).  This halves pool DMA (3.15MB -> 1.57MB per side) and
    halves the SBUF->SBUF shuffle volume.  PE does mixed bf16 x e3m4
    matmuls (verified bit-accurate on HW).
  * gate pre-activations carry the x32 factor in PSUM; the sigmoid folds
    the 1/32 into its activation scale, and the n-path PSUM is cast to bf16
    with a fused 1/32 multiply on DVE.
  * gate tail math runs in bf16 (2x DVE rate) split across DVE (t1,t2) and
    GPSIMD (d,e,out); ACT does sigmoid/tanh + 2/3 of the W casts.
  * all HBM loads ride the sync HWDGE queue (pools first, then xt/ht/hbh),
    shuffles ride the scalar HWDGE queue, hy stores go back on sync —
    nothing uses the slow gpsimd SWDGE path.

Per core the schema combine runs on the TENSOR engine via a block-diagonal
stationary: with d = dchunk*16 + dp,
  lhsT[(s,dp), (u,dp')] = sw[u,s] * delta(dp,dp')      (128x128, host-built)
  rhs [(s,dp), (dchunk,o)] = pool[s, o, d] * 32        (e3m4, streamed once)
  out [(u,dp), (dchunk,o)] = 32*W[u, d, o]             (full K=128 utilization)
PSUM chunks are cast to e3m4 SBUF (ACT/DVE), then a strided SBUF->SBUF DMA
regroups partitions (u,dp) -> d%128 per unit ("shuffle"), restoring the
natural d-order that the host-side xT layout uses.
"""

import numpy as np
import ml_dtypes

B, U, DIN, H, S = 256, 64, 256, 256, 8
NCORES = 8
UC = U // NCORES  # units per core
O3 = 3 * H        # 768
KC = DIN // 128   # 2 contraction chunks
MC = B // 128     # 2 batch chunks
DP = 16           # dp count
WSCALE = 32.0     # host-side pool scale folded out in the activations

BF16 = ml_dtypes.bfloat16
E3M4 = ml_dtypes.float8_e3m4


def _build_program():
    from contextlib import ExitStack

    import concourse.bacc as bacc
    import concourse.mybir as mybir
    import concourse.tile as tile

    bf = mybir.dt.bfloat16
    f32 = mybir.dt.float32
    e3 = mybir.dt.float8e3
    AF = mybir.ActivationFunctionType
    ALU = mybir.AluOpType

    nc = bacc.Bacc("TRN2", target_bir_lowering=False, debug=False)

    poolx = nc.dram_tensor("poolx", [128, 8, 2 * O3], e3, kind="ExternalInput")
    poolh = nc.dram_tensor("poolh", [128, 8, 2 * O3], e3, kind="ExternalInput")
    swx = nc.dram_tensor("swx", [128, 128], bf, kind="ExternalInput")
    swh = nc.dram_tensor("swh", [128, 128], bf, kind="ExternalInput")
    xt = nc.dram_tensor("xt", [128, UC, KC, B], bf, kind="ExternalInput")
    ht = nc.dram_tensor("ht", [128, UC, KC, B], bf, kind="ExternalInput")
    hbh = nc.dram_tensor("hbh", [128, UC, MC, H], bf, kind="ExternalInput")
    hy = nc.dram_tensor("hy", [128, UC, MC, H], bf, kind="ExternalOutput")

    with tile.TileContext(nc) as tc, ExitStack() as ctx:
        pconst = ctx.enter_context(tc.tile_pool(name="pconst", bufs=1))
        pgtmp = ctx.enter_context(tc.tile_pool(name="pgtmp", bufs=3))

        # --- input loads: everything on the sync HWDGE queue, in priority
        # order (sw, pools x then h, xt, ht, hbh).  The queue dispatches in
        # order so pools get the DMA engines first.
        swx_sb = pconst.tile([128, 128], bf, tag="swx")
        nc.sync.dma_start(out=swx_sb, in_=swx[:, :])
        swh_sb = pconst.tile([128, 128], bf, tag="swh")
        nc.sync.dma_start(out=swh_sb, in_=swh[:, :])
        pool_g = {}
        for t, dram in (("x", poolx), ("h", poolh)):
            for g in range(8):
                pool_g[(t, g)] = pconst.tile(
                    [128, 2 * O3], e3, tag=f"pool{t}{g}", name=f"pool{t}{g}"
                )
        for t, dram in (("x", poolx), ("h", poolh)):
            for g in range(8):
                nc.sync.dma_start(out=pool_g[(t, g)], in_=dram[:, g, :])
        xt_sb = pconst.tile([128, UC, KC, B], bf, tag="xt")
        nc.sync.dma_start(out=xt_sb, in_=xt[:, :, :, :])
        ht_sb = pconst.tile([128, UC, KC, B], bf, tag="ht")
        nc.sync.dma_start(out=ht_sb, in_=ht[:, :, :, :])
        hbh_sb = pconst.tile([128, UC, MC, H], bf, tag="hbh")
        nc.sync.dma_start(out=hbh_sb, in_=hbh[:, :, :, :])

        wsb_g = {
            (t, g): pconst.tile([128, 2 * O3], e3, tag=f"wsb{t}{g}", name=f"wsb{t}{g}")
            for t in ("x", "h")
            for g in range(8)
        }
        # all units' combined weights, gate-matmul layout: [d%128, u, d//128, o]
        wp = {
            "x": pconst.tile([128, UC, KC, O3], e3, tag="wpx", name="wpx"),
            "h": pconst.tile([128, UC, KC, O3], e3, tag="wph", name="wph"),
        }
        out_sb = {
            i: pconst.tile([128, 2, MC, H], bf, tag=f"out{i}", name=f"out{i}")
            for i in range(UC // 2)
        }

        # --- schema combine on the PE + partition-regroup shuffle ---
        # free layout of pool/wsb: (g, (kc, o)) with dchunk = kc*8 + g, so one
        # shuffle DMA per g moves 1536B-contiguous runs per source partition.
        with tc.tile_pool(name="pcomb", bufs=3, space="PSUM") as pcomb:
            for t, sw_sb in (("x", swx_sb), ("h", swh_sb)):
                for g in range(8):
                    for c in range(3):
                        os = slice(c * 512, (c + 1) * 512)
                        ps = pcomb.tile([128, 512], f32, tag="cps")
                        nc.tensor.matmul(
                            ps, sw_sb, pool_g[(t, g)][:, os],
                            start=True, stop=True,
                        )
                        dst = wsb_g[(t, g)][:, os]
                        if c == 1:
                            nc.vector.tensor_copy(out=dst, in_=ps)
                        else:
                            nc.scalar.activation(out=dst, in_=ps, func=AF.Copy)
                    # shuffle: wsb partitions are (dp,u) dp-major; src [128,
                    # (kc,o)] iterates ((dp,u), kc, o), matching the legal dst
                    # AP [16 part (dp), u, kc, o] at partition base g*16.
                    nc.scalar.dma_start(
                        out=wp[t][g * DP : (g + 1) * DP, :, :, :],
                        in_=wsb_g[(t, g)][:, :],
                    )

        # --- gate matmuls + GRU gate math ---
        INV = float(1.0 / WSCALE)
        with tc.tile_pool(name="pg", bufs=2, space="PSUM") as pg:
            for u in range(UC):
                for mc in range(MC):
                    ri = pg.tile([128, 512], f32, tag="ri", name="ri")
                    nxh = pg.tile([128, 512], f32, tag="nxh", name="nxh")
                    bs = slice(mc * 128, (mc + 1) * 128)
                    for t, t_sb, nlo in (("x", xt_sb, 0), ("h", ht_sb, 256)):
                        for kc in range(KC):
                            lhs = t_sb[:, u, kc, bs]
                            nc.tensor.matmul(
                                ri, lhs, wp[t][:, u, kc, 0:512],
                                start=(t == "x" and kc == 0),
                                stop=(t == "h" and kc == 1),
                            )
                            nc.tensor.matmul(
                                nxh[:, nlo : nlo + 256],
                                lhs, wp[t][:, u, kc, 512:O3],
                                start=(kc == 0), stop=(kc == 1),
                            )
                    # sig = [rg | ig] in bf16; 1/32 folded into the ACT scale
                    sig = pgtmp.tile([128, 512], bf, tag="sig")
                    nc.scalar.activation(out=sig, in_=ri, func=AF.Sigmoid, scale=INV)
                    # nxb = [i_n | h_n] in bf16, descaled on DVE
                    nxb = pgtmp.tile([128, 512], bf, tag="nxb")
                    nc.vector.tensor_scalar_mul(out=nxb, in0=nxh, scalar1=INV)
                    t1 = pgtmp.tile([128, H], bf, tag="t1")
                    nc.vector.tensor_tensor(
                        out=t1, in0=sig[:, 0:H], in1=nxb[:, 256:512], op=ALU.mult
                    )
                    t2 = pgtmp.tile([128, H], bf, tag="t2")
                    nc.vector.tensor_tensor(
                        out=t2, in0=t1, in1=nxb[:, 0:256], op=ALU.add
                    )
                    ng = pgtmp.tile([128, H], bf, tag="ng")
                    nc.scalar.activation(out=ng, in_=t2, func=AF.Tanh)
                    d = pgtmp.tile([128, H], bf, tag="d")
                    nc.gpsimd.tensor_tensor(
                        out=d, in0=hbh_sb[:, u, mc, :], in1=ng, op=ALU.subtract
                    )
                    e = pgtmp.tile([128, H], bf, tag="e")
                    nc.gpsimd.tensor_tensor(
                        out=e, in0=sig[:, 256:512], in1=d, op=ALU.mult
                    )
                    nc.gpsimd.tensor_tensor(
                        out=out_sb[u // 2][:, u % 2, mc, :], in0=ng, in1=e, op=ALU.add
                    )
                    if u % 2 == 1:
                        nc.sync.dma_start(
                            out=hy[:, u - 1 : u + 1, mc, :],
                            in_=out_sb[u // 2][:, :, mc, :],
                        )

    nc.compile()
    return nc


def _prep_inputs(x, hidden, pool_x, pool_h, sw_x, sw_h):
    """Host-side (free) slicing / transposition / casting per core."""
    # pool[s, o, d] -> rhs layout [(s,dp) part, (dchunk, o) free], d = dchunk*16+dp
    def prep_pool(p):
        pt = (p * WSCALE).transpose(0, 2, 1)  # [S, DIN, O3] scaled
        pt = pt.reshape(S, KC, 8, DP, O3)  # [s, kc, g, dp, o]  (d = (kc*8+g)*16+dp)
        pt = pt.transpose(0, 3, 2, 1, 4)   # [s, dp, g, kc, o]
        return np.ascontiguousarray(pt.reshape(128, 8, 2 * O3).astype(E3M4))

    poolx_h = prep_pool(pool_x)
    poolh_h = prep_pool(pool_h)

    in_maps = []
    for c in range(NCORES):
        us = slice(c * UC, (c + 1) * UC)

        def sw_block(sw_c):  # [UC, S] -> [(s,dp), (dp',u)] block-diagonal
            blk = np.zeros((S, DP, DP, UC), dtype=np.float32)
            for dp in range(DP):
                blk[:, dp, dp, :] = sw_c.T
            return np.ascontiguousarray(blk.reshape(128, 128).astype(BF16))

        xc = x[:, us, :]       # [B, UC, DIN]
        hc = hidden[:, us, :]
        # [128 (d%128), UC, KC (d//128), B]
        xt_h = np.ascontiguousarray(
            xc.transpose(1, 2, 0).reshape(UC, KC, 128, B).transpose(2, 0, 1, 3).astype(BF16)
        )
        ht_h = np.ascontiguousarray(
            hc.transpose(1, 2, 0).reshape(UC, KC, 128, B).transpose(2, 0, 1, 3).astype(BF16)
        )
        # [128 (b%128), UC, MC (b//128), H]
        hbh_h = np.ascontiguousarray(
            hc.reshape(MC, 128, UC, H).transpose(1, 2, 0, 3).astype(BF16)
        )
        in_maps.append(
            {
                "poolx": poolx_h,
                "poolh": poolh_h,
                "swx": sw_block(sw_x[us]),
                "swh": sw_block(sw_h[us]),
                "xt": xt_h,
                "ht": ht_h,
                "hbh": hbh_h,
            }
        )
    return in_maps


_CACHED_NC = None


def _get_nc():
    global _CACHED_NC
    if _CACHED_NC is None:
        _CACHED_NC = _build_program()
    return _CACHED_NC


def kernel(x, hidden, pool_x, pool_h, sw_x, sw_h, _trace=False, _results_holder=None):
    from concourse.bass_utils import run_bass_kernel_spmd

    x = np.asarray(x)
    hidden = np.asarray(hidden)
    pool_x = np.asarray(pool_x)
    pool_h = np.asarray(pool_h)
    sw_x = np.asarray(sw_x)
    sw_h = np.asarray(sw_h)

    nc = _get_nc()
    in_maps = _prep_inputs(x, hidden, pool_x, pool_h, sw_x, sw_h)
    res = run_bass_kernel_spmd(
        nc, in_maps, core_ids=list(range(NCORES)), trace=_trace
    )
    if _results_holder is not None:
        _results_holder.append(res)

    out = np.empty((B, U, H), dtype=np.float32)
    for c in range(NCORES):
        hy_c = np.asarray(res.results[c]["hy"]).astype(np.float32)  # [128, UC, MC, H]
        # out[b, u, h] with b = mc*128 + bp
        out[:, c * UC : (c + 1) * UC, :] = hy_c.transpose(2, 0, 1, 3).reshape(B, UC, H)
    return out


# revision 3
# speedup vs baseline: 1.2260x; 1.1551x over previous
"""GroupGRUCell with shared schema-pool parameters — Trainium2 Bass kernel.

Problem shapes (hardcoded): B=256 batch, U=64 GRU units, DIN=H=256, S=8 schemas.
  Wx[u] = sum_s sw_x[u,s] * pool_x[s].T   (per-unit weights from shared pool)
  gate_x = x @ Wx ; gate_h = h @ Wh ; standard GRU cell gate math.

Sharding: unit-parallel across 8 NeuronCores (8 units per core); the schema
pool is replicated per core.

v3 design:
  * pool and the combined weights W are float8 e3m4 scaled by 32 (W absmax
    ~0.24 -> ~7.6, inside e3m4 range; e2e rel-err ~1e-2 vs the 2e-2 gate).
    PE does mixed e3m4 x bf16 matmuls (verified bit-accurate on HW).
  * shuffle-free combine: the pool slice is the STATIONARY operand and a
    block-diagonal sw matrix streams, so W lands in PSUM already in the
    d%128-partition layout the gate matmuls consume:
      lhsT[(s,oc), dm] = 32 * pool[s, ob*16+oc, kc*128+dm]   (per (kc,ob))
      rhs [(s,oc), (u,oc')] = sw[u,s] * delta(oc,oc')        (constant)
      out [dm, (u,oc)] = 32 * W[u, kc*128+dm, ob*16+oc]
    K=128 fully used; 192 matmuls of 128 columns; the previous SBUF->SBUF
    partition-regroup shuffle (3.15MB of DMA) is gone entirely.
  * casts PSUM->e3m4 SBUF batch 8 matmuls (2 PSUM banks) per op and write
    through a rearranged AP into the gate weight layout; ACT/DVE alternate.
  * gate pre-activations carry the x32 factor in PSUM; sigmoid/tanh fold
    the 1/32 into their activation scale.
  * gate tail math in f32 (measured faster than bf16 on DVE): ACT does
    sigmoid/tanh, DVE does t1/t2/out, GPSIMD does d/e.
  * loads: sync HWDGE queue carries sw/pools/hbh + hy stores; scalar HWDGE
    carries xt/ht.  Nothing uses the slow gpsimd SWDGE path.
"""

import numpy as np
import ml_dtypes

B, U, DIN, H, S = 256, 64, 256, 256, 8
NCORES = 8
UC = U // NCORES  # units per core
O3 = 3 * H        # 768
KC = DIN // 128   # 2 contraction chunks
MC = B // 128     # 2 batch chunks
NOB = O3 // 16    # 48 o-blocks of 16
WSCALE = 32.0     # host-side pool scale folded out in the activations

BF16 = ml_dtypes.bfloat16
E3M4 = ml_dtypes.float8_e3m4


def _build_program():
    from contextlib import ExitStack

    import concourse.bacc as bacc
    import concourse.mybir as mybir
    import concourse.tile as tile

    bf = mybir.dt.bfloat16
    f32 = mybir.dt.float32
    e3 = mybir.dt.float8e3
    AF = mybir.ActivationFunctionType
    ALU = mybir.AluOpType

    nc = bacc.Bacc("TRN2", target_bir_lowering=False, debug=False)

    # pool in combine-lhsT layout: [(s,oc), ob, kc, dm]
    poolx = nc.dram_tensor("poolx", [128, NOB, KC, 128], e3, kind="ExternalInput")
    poolh = nc.dram_tensor("poolh", [128, NOB, KC, 128], e3, kind="ExternalInput")
    swx = nc.dram_tensor("swx", [128, 128], bf, kind="ExternalInput")
    swh = nc.dram_tensor("swh", [128, 128], bf, kind="ExternalInput")
    xt = nc.dram_tensor("xt", [128, UC, KC, B], bf, kind="ExternalInput")
    ht = nc.dram_tensor("ht", [128, UC, KC, B], bf, kind="ExternalInput")
    hbh = nc.dram_tensor("hbh", [128, UC, MC, H], bf, kind="ExternalInput")
    hy = nc.dram_tensor("hy", [128, UC, MC, H], bf, kind="ExternalOutput")

    NCH = 4           # pool DMA chunks per side
    OBC = NOB // NCH  # 12 o-blocks per chunk

    with tile.TileContext(nc) as tc, ExitStack() as ctx:
        pconst = ctx.enter_context(tc.tile_pool(name="pconst", bufs=1))
        pgtmp = ctx.enter_context(tc.tile_pool(name="pgtmp", bufs=3))

        # --- input loads ---
        # scalar HWDGE queue: xt/ht (dispatched first so they stream in
        # parallel with the pool chunks on the sync queue)
        xt_sb = pconst.tile([128, UC, KC, B], bf, tag="xt")
        nc.scalar.dma_start(out=xt_sb, in_=xt[:, :, :, :])
        ht_sb = pconst.tile([128, UC, KC, B], bf, tag="ht")
        nc.scalar.dma_start(out=ht_sb, in_=ht[:, :, :, :])
        # sync HWDGE queue: sw, pool chunks (x then h), hbh
        swx_sb = pconst.tile([128, 128], bf, tag="swx")
        nc.sync.dma_start(out=swx_sb, in_=swx[:, :])
        swh_sb = pconst.tile([128, 128], bf, tag="swh")
        nc.sync.dma_start(out=swh_sb, in_=swh[:, :])
        pool_c = {}
        for t, dram in (("x", poolx), ("h", poolh)):
            for c in range(NCH):
                pool_c[(t, c)] = pconst.tile(
                    [128, OBC, KC, 128], e3, tag=f"pool{t}{c}", name=f"pool{t}{c}"
                )
                nc.sync.dma_start(
                    out=pool_c[(t, c)], in_=dram[:, c * OBC : (c + 1) * OBC, :, :]
                )
        hbh_sb = pconst.tile([128, UC, MC, H], bf, tag="hbh")
        nc.sync.dma_start(out=hbh_sb, in_=hbh[:, :, :, :])

        # all units' combined weights, gate-matmul layout: [d%128, u, d//128, o]
        wp = {
            "x": pconst.tile([128, UC, KC, O3], e3, tag="wpx", name="wpx"),
            "h": pconst.tile([128, UC, KC, O3], e3, tag="wph", name="wph"),
        }
        out_sb = {
            i: pconst.tile([128, 2, MC, H], bf, tag=f"out{i}", name=f"out{i}")
            for i in range(UC // 2)
        }

        # --- schema combine on the PE, output directly in gate layout ---
        GRP = 8  # matmuls per cast group (2 PSUM banks)
        with tc.tile_pool(name="pcomb", bufs=2, space="PSUM") as pcomb:
            cast_rr = 0
            for t, sw_sb in (("x", swx_sb), ("h", swh_sb)):
                for kc in range(KC):
                    for obb in range(NOB // GRP):
                        ps = pcomb.tile([128, GRP, 128], f32, tag="cps")
                        for j in range(GRP):
                            ob = obb * GRP + j
                            nc.tensor.matmul(
                                ps[:, j, :],
                                pool_c[(t, ob // OBC)][:, ob % OBC, kc, :],
                                sw_sb,
                                start=True, stop=True,
                            )
                        # dst iterates (ob, u, oc) to match PSUM (j, (u,oc))
                        dst = wp[t][
                            :, :, kc, obb * GRP * 16 : (obb + 1) * GRP * 16
                        ].rearrange("p u (a b) -> p a u b", a=GRP)
                        if cast_rr % 2 == 0:
                            nc.scalar.activation(out=dst, in_=ps, func=AF.Copy)
                        else:
                            nc.vector.tensor_copy(out=dst, in_=ps)
                        cast_rr += 1

        # --- gate matmuls + GRU gate math ---
        INV = float(1.0 / WSCALE)
        with tc.tile_pool(name="pg", bufs=2, space="PSUM") as pg:
            for u in range(UC):
                for mc in range(MC):
                    ri = pg.tile([128, 512], f32, tag="ri", name="ri")
                    nxh = pg.tile([128, 512], f32, tag="nxh", name="nxh")
                    bs = slice(mc * 128, (mc + 1) * 128)
                    for t, t_sb, nlo in (("x", xt_sb, 0), ("h", ht_sb, 256)):
                        for kc in range(KC):
                            lhs = t_sb[:, u, kc, bs]
                            nc.tensor.matmul(
                                ri, lhs, wp[t][:, u, kc, 0:512],
                                start=(t == "x" and kc == 0),
                                stop=(t == "h" and kc == 1),
                            )
                            nc.tensor.matmul(
                                nxh[:, nlo : nlo + 256],
                                lhs, wp[t][:, u, kc, 512:O3],
                                start=(kc == 0), stop=(kc == 1),
                            )
                    # sig = [rg | ig]; 1/32 folded into the ACT scale
                    sig = pgtmp.tile([128, 512], f32, tag="sig")
                    nc.scalar.activation(out=sig, in_=ri, func=AF.Sigmoid, scale=INV)
                    t1 = pgtmp.tile([128, H], f32, tag="t1")
                    nc.vector.tensor_tensor(
                        out=t1, in0=sig[:, 0:H], in1=nxh[:, 256:512], op=ALU.mult
                    )
                    t2 = pgtmp.tile([128, H], f32, tag="t2")
                    nc.vector.tensor_tensor(
                        out=t2, in0=t1, in1=nxh[:, 0:256], op=ALU.add
                    )
                    # t2 still carries x32; fold 1/32 into the tanh scale
                    ng = pgtmp.tile([128, H], f32, tag="ng")
                    nc.scalar.activation(out=ng, in_=t2, func=AF.Tanh, scale=INV)
                    d = pgtmp.tile([128, H], f32, tag="d")
                    nc.gpsimd.tensor_tensor(
                        out=d, in0=hbh_sb[:, u, mc, :], in1=ng, op=ALU.subtract
                    )
                    e = pgtmp.tile([128, H], f32, tag="e")
                    nc.gpsimd.tensor_tensor(
                        out=e, in0=sig[:, 256:512], in1=d, op=ALU.mult
                    )
                    nc.vector.tensor_tensor(
                        out=out_sb[u // 2][:, u % 2, mc, :], in0=ng, in1=e, op=ALU.add
                    )
                    if u % 2 == 1:
                        nc.sync.dma_start(
                            out=hy[:, u - 1 : u + 1, mc, :],
                            in_=out_sb[u // 2][:, :, mc, :],
                        )

    nc.compile()
    return nc


def _prep_inputs(x, hidden, pool_x, pool_h, sw_x, sw_h):
    """Host-side (free) slicing / transposition / casting per core."""
    # pool[s, o, d] -> lhsT layout [(s,oc), ob, kc, dm], o = ob*16+oc, d = kc*128+dm
    def prep_pool(p):
        pt = (p * WSCALE).reshape(S, NOB, 16, KC, 128)  # [s, ob, oc, kc, dm]
        pt = pt.transpose(0, 2, 1, 3, 4)                # [s, oc, ob, kc, dm]
        return np.ascontiguousarray(pt.reshape(128, NOB, KC, 128).astype(E3M4))

    poolx_h = prep_pool(pool_x)
    poolh_h = prep_pool(pool_h)

    in_maps = []
    for c in range(NCORES):
        us = slice(c * UC, (c + 1) * UC)

        def sw_block(sw_c):  # [UC, S] -> [(s,oc), (u,oc')] block-diagonal
            blk = np.zeros((S, 16, UC, 16), dtype=np.float32)
            for oc in range(16):
                blk[:, oc, :, oc] = sw_c.T
            return np.ascontiguousarray(blk.reshape(128, 128).astype(BF16))

        xc = x[:, us, :]       # [B, UC, DIN]
        hc = hidden[:, us, :]
        # [128 (d%128), UC, KC (d//128), B]
        xt_h = np.ascontiguousarray(
            xc.transpose(1, 2, 0).reshape(UC, KC, 128, B).transpose(2, 0, 1, 3).astype(BF16)
        )
        ht_h = np.ascontiguousarray(
            hc.transpose(1, 2, 0).reshape(UC, KC, 128, B).transpose(2, 0, 1, 3).astype(BF16)
        )
        # [128 (b%128), UC, MC (b//128), H]
        hbh_h = np.ascontiguousarray(
            hc.reshape(MC, 128, UC, H).transpose(1, 2, 0, 3).astype(BF16)
        )
        in_maps.append(
            {
                "poolx": poolx_h,
                "poolh": poolh_h,
                "swx": sw_block(sw_x[us]),
                "swh": sw_block(sw_h[us]),
                "xt": xt_h,
                "ht": ht_h,
                "hbh": hbh_h,
            }
        )
    return in_maps


_CACHED_NC = None


def _get_nc():
    global _CACHED_NC
    if _CACHED_NC is None:
        _CACHED_NC = _build_program()
    return _CACHED_NC


def kernel(x, hidden, pool_x, pool_h, sw_x, sw_h, _trace=False, _results_holder=None):
    from concourse.bass_utils import run_bass_kernel_spmd

    x = np.asarray(x)
    hidden = np.asarray(hidden)
    pool_x = np.asarray(pool_x)
    pool_h = np.asarray(pool_h)
    sw_x = np.asarray(sw_x)
    sw_h = np.asarray(sw_h)

    nc = _get_nc()
    in_maps = _prep_inputs(x, hidden, pool_x, pool_h, sw_x, sw_h)
    res = run_bass_kernel_spmd(
        nc, in_maps, core_ids=list(range(NCORES)), trace=_trace
    )
    if _results_holder is not None:
        _results_holder.append(res)

    out = np.empty((B, U, H), dtype=np.float32)
    for c in range(NCORES):
        hy_c = np.asarray(res.results[c]["hy"]).astype(np.float32)  # [128, UC, MC, H]
        # out[b, u, h] with b = mc*128 + bp
        out[:, c * UC : (c + 1) * UC, :] = hy_c.transpose(2, 0, 1, 3).reshape(B, UC, H)
    return out


# revision 6
# speedup vs baseline: 1.2898x; 1.0520x over previous
"""GroupGRUCell with shared schema-pool parameters — Trainium2 Bass kernel.

Problem shapes (hardcoded): B=256 batch, U=64 GRU units, DIN=H=256, S=8 schemas.
  Wx[u] = sum_s sw_x[u,s] * pool_x[s].T   (per-unit weights from shared pool)
  gate_x = x @ Wx ; gate_h = h @ Wh ; standard GRU cell gate math.

Sharding: unit-parallel across 8 NeuronCores (8 units per core); the schema
pool is replicated per core.

v3 design:
  * pool and the combined weights W are float8 e3m4 scaled by 32 (W absmax
    ~0.24 -> ~7.6, inside e3m4 range; e2e rel-err ~1e-2 vs the 2e-2 gate).
    PE does mixed e3m4 x bf16 matmuls (verified bit-accurate on HW).
  * shuffle-free combine: the pool slice is the STATIONARY operand and a
    block-diagonal sw matrix streams, so W lands in PSUM already in the
    d%128-partition layout the gate matmuls consume:
      lhsT[(s,oc), dm] = 32 * pool[s, ob*16+oc, kc*128+dm]   (per (kc,ob))
      rhs [(s,oc), (u,oc')] = sw[u,s] * delta(oc,oc')        (constant)
      out [dm, (u,oc)] = 32 * W[u, kc*128+dm, ob*16+oc]
    K=128 fully used; 192 matmuls of 128 columns; the previous SBUF->SBUF
    partition-regroup shuffle (3.15MB of DMA) is gone entirely.
  * casts PSUM->e3m4 SBUF batch 8 matmuls (2 PSUM banks) per op and write
    through a rearranged AP into the gate weight layout; ACT/DVE alternate.
  * gate pre-activations carry the x32 factor in PSUM; sigmoid/tanh fold
    the 1/32 into their activation scale.
  * gate tail math in f32 (measured faster than bf16 on DVE): ACT does
    sigmoid/tanh, DVE does t1/t2/out, GPSIMD does d/e.
  * loads: sync HWDGE queue carries sw/pools/hbh + hy stores; scalar HWDGE
    carries xt/ht.  Nothing uses the slow gpsimd SWDGE path.
"""

import numpy as np
import ml_dtypes

B, U, DIN, H, S = 256, 64, 256, 256, 8
NCORES = 8
UC = U // NCORES  # units per core
O3 = 3 * H        # 768
KC = DIN // 128   # 2 contraction chunks
MC = B // 128     # 2 batch chunks
NOB = O3 // 16    # 48 o-blocks of 16
WSCALE = 32.0     # host-side pool scale folded out in the activations

BF16 = ml_dtypes.bfloat16
E3M4 = ml_dtypes.float8_e3m4


def _build_program():
    from contextlib import ExitStack

    import concourse.bacc as bacc
    import concourse.mybir as mybir
    import concourse.tile as tile

    bf = mybir.dt.bfloat16
    f32 = mybir.dt.float32
    e3 = mybir.dt.float8e3
    AF = mybir.ActivationFunctionType
    ALU = mybir.AluOpType

    nc = bacc.Bacc("TRN2", target_bir_lowering=False, debug=False)

    # pool in combine-lhsT layout: [(s,oc), ob, kc, dm]
    poolx = nc.dram_tensor("poolx", [128, NOB, KC, 128], e3, kind="ExternalInput")
    poolh = nc.dram_tensor("poolh", [128, NOB, KC, 128], e3, kind="ExternalInput")
    swx = nc.dram_tensor("swx", [128, 128], bf, kind="ExternalInput")
    swh = nc.dram_tensor("swh", [128, 128], bf, kind="ExternalInput")
    xt = nc.dram_tensor("xt", [128, UC, KC, B], bf, kind="ExternalInput")
    ht = nc.dram_tensor("ht", [128, UC, KC, B], bf, kind="ExternalInput")
    hbh = nc.dram_tensor("hbh", [128, UC, MC, H], bf, kind="ExternalInput")
    hy = nc.dram_tensor("hy", [128, UC, MC, H], bf, kind="ExternalOutput")

    NCH = 6           # pool DMA chunks per side
    OBC = NOB // NCH  # 8 o-blocks per chunk

    with tile.TileContext(nc) as tc, ExitStack() as ctx:
        pconst = ctx.enter_context(tc.tile_pool(name="pconst", bufs=1))
        pgtmp = ctx.enter_context(tc.tile_pool(name="pgtmp", bufs=3))

        # --- input loads ---
        # scalar HWDGE queue: xt/ht (dispatched first so they stream in
        # parallel with the pool chunks on the sync queue)
        xt_sb = pconst.tile([128, UC, KC, B], bf, tag="xt")
        nc.scalar.dma_start(out=xt_sb, in_=xt[:, :, :, :])
        ht_sb = pconst.tile([128, UC, KC, B], bf, tag="ht")
        nc.scalar.dma_start(out=ht_sb, in_=ht[:, :, :, :])
        # sync HWDGE queue: first pool-x chunk first (combine-critical), then
        # sw, the remaining pool chunks, hbh
        pool_c = {
            (t, c): pconst.tile(
                [128, OBC, KC, 128], e3, tag=f"pool{t}{c}", name=f"pool{t}{c}"
            )
            for t in ("x", "h")
            for c in range(NCH)
        }

        def load_pool(t, c):
            dram = poolx if t == "x" else poolh
            nc.sync.dma_start(
                out=pool_c[(t, c)], in_=dram[:, c * OBC : (c + 1) * OBC, :, :]
            )

        load_pool("x", 0)
        swx_sb = pconst.tile([128, 128], bf, tag="swx")
        nc.sync.dma_start(out=swx_sb, in_=swx[:, :])
        swh_sb = pconst.tile([128, 128], bf, tag="swh")
        nc.sync.dma_start(out=swh_sb, in_=swh[:, :])
        for c in range(1, NCH):
            load_pool("x", c)
        for c in range(NCH):
            load_pool("h", c)
        hbh_sb = pconst.tile([128, UC, MC, H], bf, tag="hbh")
        nc.sync.dma_start(out=hbh_sb, in_=hbh[:, :, :, :])

        # all units' combined weights, gate-matmul layout: [d%128, u, d//128, o]
        wp = {
            "x": pconst.tile([128, UC, KC, O3], e3, tag="wpx", name="wpx"),
            "h": pconst.tile([128, UC, KC, O3], e3, tag="wph", name="wph"),
        }
        out_sb = {
            i: pconst.tile([128, 2, MC, H], bf, tag=f"out{i}", name=f"out{i}")
            for i in range(UC // 2)
        }

        # --- schema combine on the PE, output directly in gate layout ---
        GRP = 8  # matmuls per cast group (2 PSUM banks)
        with tc.tile_pool(name="pcomb", bufs=2, space="PSUM") as pcomb:
            cast_rr = 0
            for t, sw_sb in (("x", swx_sb), ("h", swh_sb)):
                for kc in range(KC):
                    for obb in range(NOB // GRP):
                        ps = pcomb.tile([128, GRP, 128], f32, tag="cps")
                        for j in range(GRP):
                            ob = obb * GRP + j
                            nc.tensor.matmul(
                                ps[:, j, :],
                                pool_c[(t, ob // OBC)][:, ob % OBC, kc, :],
                                sw_sb,
                                start=True, stop=True,
                            )
                        # dst iterates (ob, u, oc) to match PSUM (j, (u,oc))
                        dst = wp[t][
                            :, :, kc, obb * GRP * 16 : (obb + 1) * GRP * 16
                        ].rearrange("p u (a b) -> p a u b", a=GRP)
                        if cast_rr % 2 == 0:
                            nc.scalar.activation(out=dst, in_=ps, func=AF.Copy)
                        else:
                            nc.vector.tensor_copy(out=dst, in_=ps)
                        cast_rr += 1

        # --- gate matmuls + GRU gate math (batched over both mc halves) ---
        INV = float(1.0 / WSCALE)
        with tc.tile_pool(name="pg", bufs=2, space="PSUM") as pg:
            for u in range(UC):
                ri = pg.tile([128, MC, 512], f32, tag="ri", name="ri")
                nxh = pg.tile([128, MC, 512], f32, tag="nxh", name="nxh")
                for mc in range(MC):
                    bs = slice(mc * 128, (mc + 1) * 128)
                    for t, t_sb, nlo in (("x", xt_sb, 0), ("h", ht_sb, 256)):
                        for kc in range(KC):
                            lhs = t_sb[:, u, kc, bs]
                            nc.tensor.matmul(
                                ri[:, mc, :], lhs, wp[t][:, u, kc, 0:512],
                                start=(t == "x" and kc == 0),
                                stop=(t == "h" and kc == 1),
                            )
                            nc.tensor.matmul(
                                nxh[:, mc, nlo : nlo + 256],
                                lhs, wp[t][:, u, kc, 512:O3],
                                start=(kc == 0), stop=(kc == 1),
                            )
                # sig = [rg | ig] per mc; 1/32 folded into the ACT scale
                sig = pgtmp.tile([128, MC, 512], f32, tag="sig")
                nc.scalar.activation(out=sig, in_=ri, func=AF.Sigmoid, scale=INV)
                t1 = pgtmp.tile([128, MC, H], f32, tag="t1")
                nc.vector.tensor_tensor(
                    out=t1, in0=sig[:, :, 0:H], in1=nxh[:, :, 256:512], op=ALU.mult
                )
                t2 = pgtmp.tile([128, MC, H], f32, tag="t2")
                nc.vector.tensor_tensor(
                    out=t2, in0=t1, in1=nxh[:, :, 0:256], op=ALU.add
                )
                # t2 still carries x32; fold 1/32 into the tanh scale
                ng = pgtmp.tile([128, MC, H], f32, tag="ng")
                nc.scalar.activation(out=ng, in_=t2, func=AF.Tanh, scale=INV)
                d = pgtmp.tile([128, MC, H], f32, tag="d")
                nc.gpsimd.tensor_tensor(
                    out=d, in0=hbh_sb[:, u, :, :], in1=ng, op=ALU.subtract
                )
                e = pgtmp.tile([128, MC, H], f32, tag="e")
                nc.gpsimd.tensor_tensor(
                    out=e, in0=sig[:, :, 256:512], in1=d, op=ALU.mult
                )
                nc.vector.tensor_tensor(
                    out=out_sb[u // 2][:, u % 2, :, :], in0=ng, in1=e, op=ALU.add
                )
                if u % 2 == 1:
                    nc.sync.dma_start(
                        out=hy[:, u - 1 : u + 1, :, :],
                        in_=out_sb[u // 2][:, :, :, :],
                    )

    nc.compile()
    return nc


def _prep_inputs(x, hidden, pool_x, pool_h, sw_x, sw_h):
    """Host-side (free) slicing / transposition / casting per core."""
    # pool[s, o, d] -> lhsT layout [(s,oc), ob, kc, dm], o = ob*16+oc, d = kc*128+dm
    def prep_pool(p):
        pt = (p * WSCALE).reshape(S, NOB, 16, KC, 128)  # [s, ob, oc, kc, dm]
        pt = pt.transpose(0, 2, 1, 3, 4)                # [s, oc, ob, kc, dm]
        return np.ascontiguousarray(pt.reshape(128, NOB, KC, 128).astype(E3M4))

    poolx_h = prep_pool(pool_x)
    poolh_h = prep_pool(pool_h)

    in_maps = []
    for c in range(NCORES):
        us = slice(c * UC, (c + 1) * UC)

        def sw_block(sw_c):  # [UC, S] -> [(s,oc), (u,oc')] block-diagonal
            blk = np.zeros((S, 16, UC, 16), dtype=np.float32)
            for oc in range(16):
                blk[:, oc, :, oc] = sw_c.T
            return np.ascontiguousarray(blk.reshape(128, 128).astype(BF16))

        xc = x[:, us, :]       # [B, UC, DIN]
        hc = hidden[:, us, :]
        # [128 (d%128), UC, KC (d//128), B]
        xt_h = np.ascontiguousarray(
            xc.transpose(1, 2, 0).reshape(UC, KC, 128, B).transpose(2, 0, 1, 3).astype(BF16)
        )
        ht_h = np.ascontiguousarray(
            hc.transpose(1, 2, 0).reshape(UC, KC, 128, B).transpose(2, 0, 1, 3).astype(BF16)
        )
        # [128 (b%128), UC, MC (b//128), H]
        hbh_h = np.ascontiguousarray(
            hc.reshape(MC, 128, UC, H).transpose(1, 2, 0, 3).astype(BF16)
        )
        in_maps.append(
            {
                "poolx": poolx_h,
                "poolh": poolh_h,
                "swx": sw_block(sw_x[us]),
                "swh": sw_block(sw_h[us]),
                "xt": xt_h,
                "ht": ht_h,
                "hbh": hbh_h,
            }
        )
    return in_maps


_CACHED_NC = None


def _get_nc():
    global _CACHED_NC
    if _CACHED_NC is None:
        _CACHED_NC = _build_program()
    return _CACHED_NC


def kernel(x, hidden, pool_x, pool_h, sw_x, sw_h, _trace=False, _results_holder=None):
    from concourse.bass_utils import run_bass_kernel_spmd

    x = np.asarray(x)
    hidden = np.asarray(hidden)
    pool_x = np.asarray(pool_x)
    pool_h = np.asarray(pool_h)
    sw_x = np.asarray(sw_x)
    sw_h = np.asarray(sw_h)

    nc = _get_nc()
    in_maps = _prep_inputs(x, hidden, pool_x, pool_h, sw_x, sw_h)
    res = run_bass_kernel_spmd(
        nc, in_maps, core_ids=list(range(NCORES)), trace=_trace
    )
    if _results_holder is not None:
        _results_holder.append(res)

    out = np.empty((B, U, H), dtype=np.float32)
    for c in range(NCORES):
        hy_c = np.asarray(res.results[c]["hy"]).astype(np.float32)  # [128, UC, MC, H]
        # out[b, u, h] with b = mc*128 + bp
        out[:, c * UC : (c + 1) * UC, :] = hy_c.transpose(2, 0, 1, 3).reshape(B, UC, H)
    return out


# revision 7
# speedup vs baseline: 1.2943x; 1.0034x over previous
"""GroupGRUCell with shared schema-pool parameters — Trainium2 Bass kernel.

Problem shapes (hardcoded): B=256 batch, U=64 GRU units, DIN=H=256, S=8 schemas.
  Wx[u] = sum_s sw_x[u,s] * pool_x[s].T   (per-unit weights from shared pool)
  gate_x = x @ Wx ; gate_h = h @ Wh ; standard GRU cell gate math.

Sharding: unit-parallel across 8 NeuronCores (8 units per core); the schema
pool is replicated per core.

v3 design:
  * pool and the combined weights W are float8 e3m4 scaled by 32 (W absmax
    ~0.24 -> ~7.6, inside e3m4 range; e2e rel-err ~1e-2 vs the 2e-2 gate).
    PE does mixed e3m4 x bf16 matmuls (verified bit-accurate on HW).
  * shuffle-free combine: the pool slice is the STATIONARY operand and a
    block-diagonal sw matrix streams, so W lands in PSUM already in the
    d%128-partition layout the gate matmuls consume:
      lhsT[(s,oc), dm] = 32 * pool[s, ob*16+oc, kc*128+dm]   (per (kc,ob))
      rhs [(s,oc), (u,oc')] = sw[u,s] * delta(oc,oc')        (constant)
      out [dm, (u,oc)] = 32 * W[u, kc*128+dm, ob*16+oc]
    K=128 fully used; 192 matmuls of 128 columns; the previous SBUF->SBUF
    partition-regroup shuffle (3.15MB of DMA) is gone entirely.
  * casts PSUM->e3m4 SBUF batch 8 matmuls (2 PSUM banks) per op and write
    through a rearranged AP into the gate weight layout; ACT/DVE alternate.
  * gate pre-activations carry the x32 factor in PSUM; sigmoid/tanh fold
    the 1/32 into their activation scale.
  * gate tail math in f32 (measured faster than bf16 on DVE): ACT does
    sigmoid/tanh, DVE does t1/t2/out, GPSIMD does d/e.
  * loads: sync HWDGE queue carries sw/pools/hbh + hy stores; scalar HWDGE
    carries xt/ht.  Nothing uses the slow gpsimd SWDGE path.
"""

import numpy as np
import ml_dtypes

B, U, DIN, H, S = 256, 64, 256, 256, 8
NCORES = 8
UC = U // NCORES  # units per core
O3 = 3 * H        # 768
KC = DIN // 128   # 2 contraction chunks
MC = B // 128     # 2 batch chunks
NOB = O3 // 16    # 48 o-blocks of 16
WSCALE = 32.0     # host-side pool scale folded out in the activations

BF16 = ml_dtypes.bfloat16
E3M4 = ml_dtypes.float8_e3m4


def _build_program():
    from contextlib import ExitStack

    import concourse.bacc as bacc
    import concourse.mybir as mybir
    import concourse.tile as tile

    bf = mybir.dt.bfloat16
    f32 = mybir.dt.float32
    e3 = mybir.dt.float8e3
    AF = mybir.ActivationFunctionType
    ALU = mybir.AluOpType

    nc = bacc.Bacc("TRN2", target_bir_lowering=False, debug=False)

    # pool in combine-lhsT layout: [(s,oc), ob, kc, dm]
    poolx = nc.dram_tensor("poolx", [128, NOB, KC, 128], e3, kind="ExternalInput")
    poolh = nc.dram_tensor("poolh", [128, NOB, KC, 128], e3, kind="ExternalInput")
    swx = nc.dram_tensor("swx", [128, 128], bf, kind="ExternalInput")
    swh = nc.dram_tensor("swh", [128, 128], bf, kind="ExternalInput")
    xt = nc.dram_tensor("xt", [128, UC, KC, B], bf, kind="ExternalInput")
    ht = nc.dram_tensor("ht", [128, UC, KC, B], bf, kind="ExternalInput")
    hbh = nc.dram_tensor("hbh", [128, UC, MC, H], bf, kind="ExternalInput")
    hy = nc.dram_tensor("hy", [128, UC, MC, H], bf, kind="ExternalOutput")

    NCH = 6           # pool DMA chunks per side
    OBC = NOB // NCH  # 8 o-blocks per chunk

    with tile.TileContext(nc) as tc, ExitStack() as ctx:
        pconst = ctx.enter_context(tc.tile_pool(name="pconst", bufs=1))
        pgtmp = ctx.enter_context(tc.tile_pool(name="pgtmp", bufs=3))

        # --- input loads ---
        # scalar HWDGE queue: xt/ht (dispatched first so they stream in
        # parallel with the pool chunks on the sync queue)
        xt_sb = pconst.tile([128, UC, KC, B], bf, tag="xt")
        nc.scalar.dma_start(out=xt_sb, in_=xt[:, :, :, :])
        ht_sb = pconst.tile([128, UC, KC, B], bf, tag="ht")
        nc.scalar.dma_start(out=ht_sb, in_=ht[:, :, :, :])
        # sync HWDGE queue: first pool-x chunk first (combine-critical), then
        # sw, the remaining pool chunks, hbh
        pool_c = {
            (t, c): pconst.tile(
                [128, OBC, KC, 128], e3, tag=f"pool{t}{c}", name=f"pool{t}{c}"
            )
            for t in ("x", "h")
            for c in range(NCH)
        }

        def load_pool(t, c):
            dram = poolx if t == "x" else poolh
            nc.sync.dma_start(
                out=pool_c[(t, c)], in_=dram[:, c * OBC : (c + 1) * OBC, :, :]
            )

        load_pool("x", 0)
        swx_sb = pconst.tile([128, 128], bf, tag="swx")
        nc.sync.dma_start(out=swx_sb, in_=swx[:, :])
        swh_sb = pconst.tile([128, 128], bf, tag="swh")
        nc.sync.dma_start(out=swh_sb, in_=swh[:, :])
        for c in range(1, NCH):
            load_pool("x", c)
        for c in range(NCH):
            load_pool("h", c)
        hbh_sb = pconst.tile([128, UC, MC, H], bf, tag="hbh")
        nc.sync.dma_start(out=hbh_sb, in_=hbh[:, :, :, :])

        # all units' combined weights, gate-matmul layout: [d%128, u, d//128, o]
        wp = {
            "x": pconst.tile([128, UC, KC, O3], e3, tag="wpx", name="wpx"),
            "h": pconst.tile([128, UC, KC, O3], e3, tag="wph", name="wph"),
        }
        out_sb = {
            i: pconst.tile([128, 2, MC, H], bf, tag=f"out{i}", name=f"out{i}")
            for i in range(UC // 2)
        }

        # --- schema combine on the PE, output directly in gate layout ---
        GRP = 8  # matmuls per cast group (2 PSUM banks)
        with tc.tile_pool(name="pcomb", bufs=2, space="PSUM") as pcomb:
            cast_rr = 0
            for t, sw_sb in (("x", swx_sb), ("h", swh_sb)):
                for kc in range(KC):
                    for obb in range(NOB // GRP):
                        ps = pcomb.tile([128, GRP, 128], f32, tag="cps")
                        for j in range(GRP):
                            ob = obb * GRP + j
                            nc.tensor.matmul(
                                ps[:, j, :],
                                pool_c[(t, ob // OBC)][:, ob % OBC, kc, :],
                                sw_sb,
                                start=True, stop=True,
                            )
                        # dst iterates (ob, u, oc) to match PSUM (j, (u,oc))
                        dst = wp[t][
                            :, :, kc, obb * GRP * 16 : (obb + 1) * GRP * 16
                        ].rearrange("p u (a b) -> p a u b", a=GRP)
                        if cast_rr % 2 == 0:
                            nc.scalar.activation(out=dst, in_=ps, func=AF.Copy)
                        else:
                            nc.vector.tensor_copy(out=dst, in_=ps)
                        cast_rr += 1

        # --- gate matmuls + GRU gate math (batched over both mc halves) ---
        # Elementwise work is software-pipelined with a one-unit skew so no
        # engine's in-order queue blocks on a cross-engine dependency:
        #   ACT: sig(u), tanh(u-1)   DVE: t1(u), t2(u), out(u-1)
        #   GPSIMD: d(u-1), e(u-1)   sync: hy store (u-1 pair)
        INV = float(1.0 / WSCALE)
        stage2 = {}  # u -> (sig, nxh-derived tiles) for the skewed back half

        def emit_front(u, pg):
            ri = pg.tile([128, MC, 512], f32, tag="ri", name="ri")
            nxh = pg.tile([128, MC, 512], f32, tag="nxh", name="nxh")
            for mc in range(MC):
                bs = slice(mc * 128, (mc + 1) * 128)
                for t, t_sb, nlo in (("x", xt_sb, 0), ("h", ht_sb, 256)):
                    for kc in range(KC):
                        lhs = t_sb[:, u, kc, bs]
                        nc.tensor.matmul(
                            ri[:, mc, :], lhs, wp[t][:, u, kc, 0:512],
                            start=(t == "x" and kc == 0),
                            stop=(t == "h" and kc == 1),
                        )
                        nc.tensor.matmul(
                            nxh[:, mc, nlo : nlo + 256],
                            lhs, wp[t][:, u, kc, 512:O3],
                            start=(kc == 0), stop=(kc == 1),
                        )
            # sig = [rg | ig] per mc; 1/32 folded into the ACT scale
            sig = pgtmp.tile([128, MC, 512], f32, tag="sig")
            nc.scalar.activation(out=sig, in_=ri, func=AF.Sigmoid, scale=INV)
            t1 = pgtmp.tile([128, MC, H], f32, tag="t1")
            nc.vector.tensor_tensor(
                out=t1, in0=sig[:, :, 0:H], in1=nxh[:, :, 256:512], op=ALU.mult
            )
            t2 = pgtmp.tile([128, MC, H], f32, tag="t2")
            nc.vector.tensor_tensor(
                out=t2, in0=t1, in1=nxh[:, :, 0:256], op=ALU.add
            )
            stage2[u] = (sig, t2)

        def emit_back(u):
            sig, t2 = stage2.pop(u)
            # t2 still carries x32; fold 1/32 into the tanh scale
            ng = pgtmp.tile([128, MC, H], f32, tag="ng")
            nc.scalar.activation(out=ng, in_=t2, func=AF.Tanh, scale=INV)
            d = pgtmp.tile([128, MC, H], f32, tag="d")
            nc.gpsimd.tensor_tensor(
                out=d, in0=hbh_sb[:, u, :, :], in1=ng, op=ALU.subtract
            )
            e = pgtmp.tile([128, MC, H], f32, tag="e")
            nc.gpsimd.tensor_tensor(
                out=e, in0=sig[:, :, 256:512], in1=d, op=ALU.mult
            )
            nc.vector.tensor_tensor(
                out=out_sb[u // 2][:, u % 2, :, :], in0=ng, in1=e, op=ALU.add
            )
            if u % 2 == 1:
                nc.sync.dma_start(
                    out=hy[:, u - 1 : u + 1, :, :],
                    in_=out_sb[u // 2][:, :, :, :],
                )

        with tc.tile_pool(name="pg", bufs=2, space="PSUM") as pg:
            for u in range(UC):
                emit_front(u, pg)
                if u >= 1:
                    emit_back(u - 1)
            emit_back(UC - 1)

    nc.compile()
    return nc


def _prep_inputs(x, hidden, pool_x, pool_h, sw_x, sw_h):
    """Host-side (free) slicing / transposition / casting per core."""
    # pool[s, o, d] -> lhsT layout [(s,oc), ob, kc, dm], o = ob*16+oc, d = kc*128+dm
    def prep_pool(p):
        pt = (p * WSCALE).reshape(S, NOB, 16, KC, 128)  # [s, ob, oc, kc, dm]
        pt = pt.transpose(0, 2, 1, 3, 4)                # [s, oc, ob, kc, dm]
        return np.ascontiguousarray(pt.reshape(128, NOB, KC, 128).astype(E3M4))

    poolx_h = prep_pool(pool_x)
    poolh_h = prep_pool(pool_h)

    in_maps = []
    for c in range(NCORES):
        us = slice(c * UC, (c + 1) * UC)

        def sw_block(sw_c):  # [UC, S] -> [(s,oc), (u,oc')] block-diagonal
            blk = np.zeros((S, 16, UC, 16), dtype=np.float32)
            for oc in range(16):
                blk[:, oc, :, oc] = sw_c.T
            return np.ascontiguousarray(blk.reshape(128, 128).astype(BF16))

        xc = x[:, us, :]       # [B, UC, DIN]
        hc = hidden[:, us, :]
        # [128 (d%128), UC, KC (d//128), B]
        xt_h = np.ascontiguousarray(
            xc.transpose(1, 2, 0).reshape(UC, KC, 128, B).transpose(2, 0, 1, 3).astype(BF16)
        )
        ht_h = np.ascontiguousarray(
            hc.transpose(1, 2, 0).reshape(UC, KC, 128, B).transpose(2, 0, 1, 3).astype(BF16)
        )
        # [128 (b%128), UC, MC (b//128), H]
        hbh_h = np.ascontiguousarray(
            hc.reshape(MC, 128, UC, H).transpose(1, 2, 0, 3).astype(BF16)
        )
        in_maps.append(
            {
                "poolx": poolx_h,
                "poolh": poolh_h,
                "swx": sw_block(sw_x[us]),
                "swh": sw_block(sw_h[us]),
                "xt": xt_h,
                "ht": ht_h,
                "hbh": hbh_h,
            }
        )
    return in_maps


_CACHED_NC = None


def _get_nc():
    global _CACHED_NC
    if _CACHED_NC is None:
        _CACHED_NC = _build_program()
    return _CACHED_NC


def kernel(x, hidden, pool_x, pool_h, sw_x, sw_h, _trace=False, _results_holder=None):
    from concourse.bass_utils import run_bass_kernel_spmd

    x = np.asarray(x)
    hidden = np.asarray(hidden)
    pool_x = np.asarray(pool_x)
    pool_h = np.asarray(pool_h)
    sw_x = np.asarray(sw_x)
    sw_h = np.asarray(sw_h)

    nc = _get_nc()
    in_maps = _prep_inputs(x, hidden, pool_x, pool_h, sw_x, sw_h)
    res = run_bass_kernel_spmd(
        nc, in_maps, core_ids=list(range(NCORES)), trace=_trace
    )
    if _results_holder is not None:
        _results_holder.append(res)

    out = np.empty((B, U, H), dtype=np.float32)
    for c in range(NCORES):
        hy_c = np.asarray(res.results[c]["hy"]).astype(np.float32)  # [128, UC, MC, H]
        # out[b, u, h] with b = mc*128 + bp
        out[:, c * UC : (c + 1) * UC, :] = hy_c.transpose(2, 0, 1, 3).reshape(B, UC, H)
    return out


# revision 9
# speedup vs baseline: 1.4141x; 1.0926x over previous
"""GroupGRUCell with shared schema-pool parameters — Trainium2 Bass kernel.

Problem shapes (hardcoded): B=256 batch, U=64 GRU units, DIN=H=256, S=8 schemas.
  Wx[u] = sum_s sw_x[u,s] * pool_x[s].T   (per-unit weights from shared pool)
  gate_x = x @ Wx ; gate_h = h @ Wh ; standard GRU cell gate math.

Sharding: unit-parallel across 8 NeuronCores (8 units per core); the schema
pool is replicated per core.

v3 design:
  * pool and the combined weights W are float8 e3m4 scaled by 32 (W absmax
    ~0.24 -> ~7.6, inside e3m4 range; e2e rel-err ~1e-2 vs the 2e-2 gate).
    PE does mixed e3m4 x bf16 matmuls (verified bit-accurate on HW).
  * shuffle-free combine: the pool slice is the STATIONARY operand and a
    block-diagonal sw matrix streams, so W lands in PSUM already in the
    d%128-partition layout the gate matmuls consume:
      lhsT[(s,oc), dm] = 32 * pool[s, ob*16+oc, kc*128+dm]   (per (kc,ob))
      rhs [(s,oc), (u,oc')] = sw[u,s] * delta(oc,oc')        (constant)
      out [dm, (u,oc)] = 32 * W[u, kc*128+dm, ob*16+oc]
    K=128 fully used; 192 matmuls of 128 columns; the previous SBUF->SBUF
    partition-regroup shuffle (3.15MB of DMA) is gone entirely.
  * casts PSUM->e3m4 SBUF batch 8 matmuls (2 PSUM banks) per op and write
    through a rearranged AP into the gate weight layout; ACT/DVE alternate.
  * gate pre-activations carry the x32 factor in PSUM; sigmoid/tanh fold
    the 1/32 into their activation scale.
  * gate tail math in f32 (measured faster than bf16 on DVE): ACT does
    sigmoid/tanh, DVE does t1/t2/out, GPSIMD does d/e.
  * loads: sync HWDGE queue carries sw/pools/hbh + hy stores; scalar HWDGE
    carries xt/ht.  Nothing uses the slow gpsimd SWDGE path.
"""

import numpy as np
import ml_dtypes

B, U, DIN, H, S = 256, 64, 256, 256, 8
NCORES = 8
UC = U // NCORES  # units per core
O3 = 3 * H        # 768
KC = DIN // 128   # 2 contraction chunks
MC = B // 128     # 2 batch chunks
NOB = O3 // 16    # 48 o-blocks of 16
WSCALE = 32.0     # host-side pool scale folded out in the activations

BF16 = ml_dtypes.bfloat16
E3M4 = ml_dtypes.float8_e3m4


def _build_program():
    from contextlib import ExitStack

    import concourse.bacc as bacc
    import concourse.mybir as mybir
    import concourse.tile as tile

    bf = mybir.dt.bfloat16
    f32 = mybir.dt.float32
    e3 = mybir.dt.float8e3
    AF = mybir.ActivationFunctionType
    ALU = mybir.AluOpType

    nc = bacc.Bacc("TRN2", target_bir_lowering=False, debug=False)

    # pool in combine-lhsT layout: [(s,oc), ob, kc, dm]
    poolx = nc.dram_tensor("poolx", [128, NOB, KC, 128], e3, kind="ExternalInput")
    poolh = nc.dram_tensor("poolh", [128, NOB, KC, 128], e3, kind="ExternalInput")
    swx = nc.dram_tensor("swx", [128, 128], bf, kind="ExternalInput")
    swh = nc.dram_tensor("swh", [128, 128], bf, kind="ExternalInput")
    xt = nc.dram_tensor("xt", [128, UC, KC, B], bf, kind="ExternalInput")
    ht = nc.dram_tensor("ht", [128, UC, KC, B], bf, kind="ExternalInput")
    hbh = nc.dram_tensor("hbh", [128, UC, MC, H], bf, kind="ExternalInput")
    hy = nc.dram_tensor("hy", [128, UC, MC, H], bf, kind="ExternalOutput")

    NCH = 6           # pool DMA chunks per side
    OBC = NOB // NCH  # 8 o-blocks per chunk

    with tile.TileContext(nc) as tc, ExitStack() as ctx:
        pconst = ctx.enter_context(tc.tile_pool(name="pconst", bufs=1))
        pgtmp = ctx.enter_context(tc.tile_pool(name="pgtmp", bufs=3))

        # --- input loads ---
        # scalar HWDGE queue: xt/ht (dispatched first so they stream in
        # parallel with the pool chunks on the sync queue)
        xt_sb = pconst.tile([128, UC, KC, B], bf, tag="xt")
        nc.scalar.dma_start(out=xt_sb, in_=xt[:, :, :, :])
        ht_sb = pconst.tile([128, UC, KC, B], bf, tag="ht")
        nc.scalar.dma_start(out=ht_sb, in_=ht[:, :, :, :])
        # sync HWDGE queue: first pool-x chunk first (combine-critical), then
        # sw, the remaining pool chunks, hbh
        pool_c = {
            (t, c): pconst.tile(
                [128, OBC, KC, 128], e3, tag=f"pool{t}{c}", name=f"pool{t}{c}"
            )
            for t in ("x", "h")
            for c in range(NCH)
        }

        def load_pool(t, c):
            dram = poolx if t == "x" else poolh
            nc.sync.dma_start(
                out=pool_c[(t, c)], in_=dram[:, c * OBC : (c + 1) * OBC, :, :]
            )

        load_pool("x", 0)
        swx_sb = pconst.tile([128, 128], bf, tag="swx")
        nc.sync.dma_start(out=swx_sb, in_=swx[:, :])
        swh_sb = pconst.tile([128, 128], bf, tag="swh")
        nc.sync.dma_start(out=swh_sb, in_=swh[:, :])
        for c in range(1, NCH):
            load_pool("x", c)
        for c in range(NCH):
            load_pool("h", c)
        hbh_sb = pconst.tile([128, UC, MC, H], bf, tag="hbh")
        nc.sync.dma_start(out=hbh_sb, in_=hbh[:, :, :, :])

        # warm the ACT sigmoid/tanh tables during startup so the table load
        # (2x ~1.3us) doesn't sit between the combine casts and the first
        # real sigmoid
        warm = pconst.tile([128, 2], f32, tag="warm")
        nc.scalar.activation(out=warm[:, 0:1], in_=swx_sb[:, 0:1], func=AF.Sigmoid)
        nc.scalar.activation(out=warm[:, 1:2], in_=swx_sb[:, 0:1], func=AF.Tanh)

        # all units' combined weights, gate-matmul layout: [d%128, u, d//128, o]
        wp = {
            "x": pconst.tile([128, UC, KC, O3], e3, tag="wpx", name="wpx"),
            "h": pconst.tile([128, UC, KC, O3], e3, tag="wph", name="wph"),
        }
        out_sb = {
            i: pconst.tile([128, 2, MC, H], bf, tag=f"out{i}", name=f"out{i}")
            for i in range(UC // 2)
        }

        # --- schema combine on the PE, output directly in gate layout ---
        GRP = 8  # matmuls per cast group (2 PSUM banks)
        with tc.tile_pool(name="pcomb", bufs=3, space="PSUM") as pcomb:
            cast_rr = 0
            for t, sw_sb in (("x", swx_sb), ("h", swh_sb)):
                for kc in range(KC):
                    for obb in range(NOB // GRP):
                        ps = pcomb.tile([128, GRP, 128], f32, tag="cps")
                        for j in range(GRP):
                            ob = obb * GRP + j
                            nc.tensor.matmul(
                                ps[:, j, :],
                                pool_c[(t, ob // OBC)][:, ob % OBC, kc, :],
                                sw_sb,
                                start=True, stop=True,
                            )
                        # dst iterates (ob, u, oc) to match PSUM (j, (u,oc))
                        dst = wp[t][
                            :, :, kc, obb * GRP * 16 : (obb + 1) * GRP * 16
                        ].rearrange("p u (a b) -> p a u b", a=GRP)
                        if cast_rr % 2 == 0:
                            nc.scalar.activation(out=dst, in_=ps, func=AF.Copy)
                        else:
                            nc.vector.tensor_copy(out=dst, in_=ps)
                        cast_rr += 1

        # --- gate matmuls + GRU gate math (batched over both mc halves) ---
        # Elementwise work is software-pipelined with a one-unit skew so no
        # engine's in-order queue blocks on a cross-engine dependency:
        #   ACT: sig(u), tanh(u-1)   DVE: t1(u), t2(u), out(u-1)
        #   GPSIMD: d(u-1), e(u-1)   sync: hy store (u-1 pair)
        INV = float(1.0 / WSCALE)
        stage2 = {}  # u -> (sig, nxh-derived tiles) for the skewed back half

        def emit_front(u, pg):
            ri = pg.tile([128, MC, 512], f32, tag="ri", name="ri")
            nxh = pg.tile([128, MC, 512], f32, tag="nxh", name="nxh")
            for mc in range(MC):
                bs = slice(mc * 128, (mc + 1) * 128)
                for t, t_sb, nlo in (("x", xt_sb, 0), ("h", ht_sb, 256)):
                    for kc in range(KC):
                        lhs = t_sb[:, u, kc, bs]
                        nc.tensor.matmul(
                            ri[:, mc, :], lhs, wp[t][:, u, kc, 0:512],
                            start=(t == "x" and kc == 0),
                            stop=(t == "h" and kc == 1),
                        )
                        nc.tensor.matmul(
                            nxh[:, mc, nlo : nlo + 256],
                            lhs, wp[t][:, u, kc, 512:O3],
                            start=(kc == 0), stop=(kc == 1),
                        )
            # sig = [rg | ig] per mc; 1/32 folded into the ACT scale
            sig = pgtmp.tile([128, MC, 512], f32, tag="sig")
            nc.scalar.activation(out=sig, in_=ri, func=AF.Sigmoid, scale=INV)
            t1 = pgtmp.tile([128, MC, H], f32, tag="t1")
            nc.vector.tensor_tensor(
                out=t1, in0=sig[:, :, 0:H], in1=nxh[:, :, 256:512], op=ALU.mult
            )
            t2 = pgtmp.tile([128, MC, H], f32, tag="t2")
            nc.vector.tensor_tensor(
                out=t2, in0=t1, in1=nxh[:, :, 0:256], op=ALU.add
            )
            stage2[u] = (sig, t2)

        def emit_back(u):
            sig, t2 = stage2.pop(u)
            # t2 still carries x32; fold 1/32 into the tanh scale
            ng = pgtmp.tile([128, MC, H], f32, tag="ng")
            nc.scalar.activation(out=ng, in_=t2, func=AF.Tanh, scale=INV)
            d = pgtmp.tile([128, MC, H], f32, tag="d")
            nc.gpsimd.tensor_tensor(
                out=d, in0=hbh_sb[:, u, :, :], in1=ng, op=ALU.subtract
            )
            e = pgtmp.tile([128, MC, H], f32, tag="e")
            nc.gpsimd.tensor_tensor(
                out=e, in0=sig[:, :, 256:512], in1=d, op=ALU.mult
            )
            nc.vector.tensor_tensor(
                out=out_sb[u // 2][:, u % 2, :, :], in0=ng, in1=e, op=ALU.add
            )
            if u % 2 == 1:
                nc.sync.dma_start(
                    out=hy[:, u - 1 : u + 1, :, :],
                    in_=out_sb[u // 2][:, :, :, :],
                )

        with tc.tile_pool(name="pg", bufs=2, space="PSUM") as pg:
            for u in range(UC):
                emit_front(u, pg)
                if u >= 1:
                    emit_back(u - 1)
            emit_back(UC - 1)

    nc.compile()
    return nc


def _prep_inputs(x, hidden, pool_x, pool_h, sw_x, sw_h):
    """Host-side (free) slicing / transposition / casting per core."""
    # pool[s, o, d] -> lhsT layout [(s,oc), ob, kc, dm], o = ob*16+oc, d = kc*128+dm
    def prep_pool(p):
        pt = (p * WSCALE).reshape(S, NOB, 16, KC, 128)  # [s, ob, oc, kc, dm]
        pt = pt.transpose(0, 2, 1, 3, 4)                # [s, oc, ob, kc, dm]
        return np.ascontiguousarray(pt.reshape(128, NOB, KC, 128).astype(E3M4))

    poolx_h = prep_pool(pool_x)
    poolh_h = prep_pool(pool_h)

    in_maps = []
    for c in range(NCORES):
        us = slice(c * UC, (c + 1) * UC)

        def sw_block(sw_c):  # [UC, S] -> [(s,oc), (u,oc')] block-diagonal
            blk = np.zeros((S, 16, UC, 16), dtype=np.float32)
            for oc in range(16):
                blk[:, oc, :, oc] = sw_c.T
            return np.ascontiguousarray(blk.reshape(128, 128).astype(BF16))

        xc = x[:, us, :]       # [B, UC, DIN]
        hc = hidden[:, us, :]
        # [128 (d%128), UC, KC (d//128), B]
        xt_h = np.ascontiguousarray(
            xc.transpose(1, 2, 0).reshape(UC, KC, 128, B).transpose(2, 0, 1, 3).astype(BF16)
        )
        ht_h = np.ascontiguousarray(
            hc.transpose(1, 2, 0).reshape(UC, KC, 128, B).transpose(2, 0, 1, 3).astype(BF16)
        )
        # [128 (b%128), UC, MC (b//128), H]
        hbh_h = np.ascontiguousarray(
            hc.reshape(MC, 128, UC, H).transpose(1, 2, 0, 3).astype(BF16)
        )
        in_maps.append(
            {
                "poolx": poolx_h,
                "poolh": poolh_h,
                "swx": sw_block(sw_x[us]),
                "swh": sw_block(sw_h[us]),
                "xt": xt_h,
                "ht": ht_h,
                "hbh": hbh_h,
            }
        )
    return in_maps


_CACHED_NC = None


def _get_nc():
    global _CACHED_NC
    if _CACHED_NC is None:
        _CACHED_NC = _build_program()
    return _CACHED_NC


def kernel(x, hidden, pool_x, pool_h, sw_x, sw_h, _trace=False, _results_holder=None):
    from concourse.bass_utils import run_bass_kernel_spmd

    x = np.asarray(x)
    hidden = np.asarray(hidden)
    pool_x = np.asarray(pool_x)
    pool_h = np.asarray(pool_h)
    sw_x = np.asarray(sw_x)
    sw_h = np.asarray(sw_h)

    nc = _get_nc()
    in_maps = _prep_inputs(x, hidden, pool_x, pool_h, sw_x, sw_h)
    res = run_bass_kernel_spmd(
        nc, in_maps, core_ids=list(range(NCORES)), trace=_trace
    )
    if _results_holder is not None:
        _results_holder.append(res)

    out = np.empty((B, U, H), dtype=np.float32)
    for c in range(NCORES):
        hy_c = np.asarray(res.results[c]["hy"]).astype(np.float32)  # [128, UC, MC, H]
        # out[b, u, h] with b = mc*128 + bp
        out[:, c * UC : (c + 1) * UC, :] = hy_c.transpose(2, 0, 1, 3).reshape(B, UC, H)
    return out


# revision 12
# speedup vs baseline: 1.4596x; 1.0321x over previous
"""GroupGRUCell with shared schema-pool parameters — Trainium2 Bass kernel.

Problem shapes (hardcoded): B=256 batch, U=64 GRU units, DIN=H=256, S=8 schemas.
  Wx[u] = sum_s sw_x[u,s] * pool_x[s].T   (per-unit weights from shared pool)
  gate_x = x @ Wx ; gate_h = h @ Wh ; standard GRU cell gate math.

Sharding: unit-parallel across 8 NeuronCores (8 units per core); the schema
pool is replicated per core.

v3 design:
  * pool and the combined weights W are float8 e3m4 scaled by 32 (W absmax
    ~0.24 -> ~7.6, inside e3m4 range; e2e rel-err ~1e-2 vs the 2e-2 gate).
    PE does mixed e3m4 x bf16 matmuls (verified bit-accurate on HW).
  * shuffle-free combine: the pool slice is the STATIONARY operand and a
    block-diagonal sw matrix streams, so W lands in PSUM already in the
    d%128-partition layout the gate matmuls consume:
      lhsT[(s,oc), dm] = 32 * pool[s, ob*16+oc, kc*128+dm]   (per (kc,ob))
      rhs [(s,oc), (u,oc')] = sw[u,s] * delta(oc,oc')        (constant)
      out [dm, (u,oc)] = 32 * W[u, kc*128+dm, ob*16+oc]
    K=128 fully used; 192 matmuls of 128 columns; the previous SBUF->SBUF
    partition-regroup shuffle (3.15MB of DMA) is gone entirely.
  * casts PSUM->e3m4 SBUF batch 8 matmuls (2 PSUM banks) per op and write
    through a rearranged AP into the gate weight layout; ACT/DVE alternate.
  * gate pre-activations carry the x32 factor in PSUM; sigmoid/tanh fold
    the 1/32 into their activation scale.
  * gate tail math in f32 (measured faster than bf16 on DVE): ACT does
    sigmoid/tanh, DVE does t1/t2/out, GPSIMD does d/e.
  * loads: sync HWDGE queue carries sw/pools/hbh + hy stores; scalar HWDGE
    carries xt/ht.  Nothing uses the slow gpsimd SWDGE path.
"""

import numpy as np
import ml_dtypes

B, U, DIN, H, S = 256, 64, 256, 256, 8
NCORES = 8
UC = U // NCORES  # units per core
O3 = 3 * H        # 768
KC = DIN // 128   # 2 contraction chunks
MC = B // 128     # 2 batch chunks
NOB = O3 // 16    # 48 o-blocks of 16
WSCALE = 32.0     # host-side pool scale folded out in the activations

BF16 = ml_dtypes.bfloat16
E3M4 = ml_dtypes.float8_e3m4


def _build_program():
    from contextlib import ExitStack

    import concourse.bacc as bacc
    import concourse.mybir as mybir
    import concourse.tile as tile

    bf = mybir.dt.bfloat16
    f32 = mybir.dt.float32
    e3 = mybir.dt.float8e3
    AF = mybir.ActivationFunctionType
    ALU = mybir.AluOpType

    nc = bacc.Bacc("TRN2", target_bir_lowering=False, debug=False)

    # pool in combine-lhsT layout: [(s,oc), ob, kc, dm]
    poolx = nc.dram_tensor("poolx", [128, NOB, KC, 128], e3, kind="ExternalInput")
    poolh = nc.dram_tensor("poolh", [128, NOB, KC, 128], e3, kind="ExternalInput")
    swx = nc.dram_tensor("swx", [128, 128], bf, kind="ExternalInput")
    swh = nc.dram_tensor("swh", [128, 128], bf, kind="ExternalInput")
    xt = nc.dram_tensor("xt", [128, UC, KC, B], bf, kind="ExternalInput")
    ht = nc.dram_tensor("ht", [128, UC, KC, B], bf, kind="ExternalInput")
    hbh = nc.dram_tensor("hbh", [128, UC, MC, H], bf, kind="ExternalInput")
    hy = nc.dram_tensor("hy", [128, UC, MC, H], bf, kind="ExternalOutput")

    NCH = 6           # pool DMA chunks per side
    OBC = NOB // NCH  # 8 o-blocks per chunk

    with tile.TileContext(nc) as tc, ExitStack() as ctx:
        pconst = ctx.enter_context(tc.tile_pool(name="pconst", bufs=1))
        pgtmp = ctx.enter_context(tc.tile_pool(name="pgtmp", bufs=3))

        # --- input loads ---
        # pool chunks split across both HWDGE queues: pool-x on sync (paces
        # the first combine sweep), pool-h on scalar.  xt/ht/hbh follow the
        # pool-x chunks on sync — gates don't need them until ~30us in.
        pool_c = {
            (t, c): pconst.tile(
                [128, OBC, KC, 128], e3, tag=f"pool{t}{c}", name=f"pool{t}{c}"
            )
            for t in ("x", "h")
            for c in range(NCH)
        }

        def load_pool(t, c, eng):
            dram = poolx if t == "x" else poolh
            eng.dma_start(
                out=pool_c[(t, c)], in_=dram[:, c * OBC : (c + 1) * OBC, :, :]
            )

        load_pool("x", 0, nc.sync)
        swx_sb = pconst.tile([128, 128], bf, tag="swx")
        nc.sync.dma_start(out=swx_sb, in_=swx[:, :])
        swh_sb = pconst.tile([128, 128], bf, tag="swh")
        nc.sync.dma_start(out=swh_sb, in_=swh[:, :])
        for c in range(NCH):
            load_pool("h", c, nc.scalar)
        for c in range(1, NCH):
            load_pool("x", c, nc.sync)
        xt_sb = pconst.tile([128, UC, KC, B], bf, tag="xt")
        nc.sync.dma_start(out=xt_sb, in_=xt[:, :, :, :])
        ht_sb = pconst.tile([128, UC, KC, B], bf, tag="ht")
        nc.sync.dma_start(out=ht_sb, in_=ht[:, :, :, :])
        hbh_sb = pconst.tile([128, UC, MC, H], bf, tag="hbh")
        nc.sync.dma_start(out=hbh_sb, in_=hbh[:, :, :, :])

        # warm the ACT sigmoid/tanh tables during startup so the table load
        # (2x ~1.3us) doesn't sit between the combine casts and the first
        # real sigmoid
        warm = pconst.tile([128, 2], f32, tag="warm")
        nc.scalar.activation(out=warm[:, 0:1], in_=swx_sb[:, 0:1], func=AF.Sigmoid)
        nc.scalar.activation(out=warm[:, 1:2], in_=swx_sb[:, 0:1], func=AF.Tanh)

        # all units' combined weights, gate-matmul layout: [d%128, u, d//128, o]
        wp = {
            "x": pconst.tile([128, UC, KC, O3], e3, tag="wpx", name="wpx"),
            "h": pconst.tile([128, UC, KC, O3], e3, tag="wph", name="wph"),
        }
        out_sb = {
            i: pconst.tile([128, 2, MC, H], bf, tag=f"out{i}", name=f"out{i}")
            for i in range(UC // 2)
        }

        # --- schema combine on the PE, output directly in gate layout ---
        GRP = 8  # matmuls per cast group (2 PSUM banks)
        with tc.tile_pool(name="pcomb", bufs=3, space="PSUM") as pcomb:
            cast_rr = 0
            for t, sw_sb in (("x", swx_sb), ("h", swh_sb)):
                # obb-outer so each arriving pool chunk is consumed (both kc)
                # before the next is needed — keeps PE paced by DMA ramp only
                # at the very start
                for obb in range(NOB // GRP):
                    for kc in range(KC):
                        ps = pcomb.tile([128, GRP, 128], f32, tag="cps")
                        for j in range(GRP):
                            ob = obb * GRP + j
                            nc.tensor.matmul(
                                ps[:, j, :],
                                pool_c[(t, ob // OBC)][:, ob % OBC, kc, :],
                                sw_sb,
                                start=True, stop=True,
                            )
                        # dst iterates (ob, u, oc) to match PSUM (j, (u,oc))
                        dst = wp[t][
                            :, :, kc, obb * GRP * 16 : (obb + 1) * GRP * 16
                        ].rearrange("p u (a b) -> p a u b", a=GRP)
                        if cast_rr % 2 == 0:
                            nc.scalar.activation(out=dst, in_=ps, func=AF.Copy)
                        else:
                            nc.vector.tensor_copy(out=dst, in_=ps)
                        cast_rr += 1

        # --- gate matmuls + GRU gate math (batched over both mc halves) ---
        # Elementwise work is software-pipelined with a one-unit skew so no
        # engine's in-order queue blocks on a cross-engine dependency:
        #   ACT: sig(u), tanh(u-1)   DVE: t1(u), t2(u), out(u-1)
        #   GPSIMD: d(u-1), e(u-1)   sync: hy store (u-1 pair)
        INV = float(1.0 / WSCALE)
        stage2 = {}  # u -> (sig, nxh-derived tiles) for the skewed back half

        def emit_front(u, pg):
            ri = pg.tile([128, MC, 512], f32, tag="ri", name="ri")
            nxh = pg.tile([128, MC, 512], f32, tag="nxh", name="nxh")
            for mc in range(MC):
                bs = slice(mc * 128, (mc + 1) * 128)
                for t, t_sb, nlo in (("x", xt_sb, 0), ("h", ht_sb, 256)):
                    for kc in range(KC):
                        lhs = t_sb[:, u, kc, bs]
                        nc.tensor.matmul(
                            ri[:, mc, :], lhs, wp[t][:, u, kc, 0:512],
                            start=(t == "x" and kc == 0),
                            stop=(t == "h" and kc == 1),
                        )
                        nc.tensor.matmul(
                            nxh[:, mc, nlo : nlo + 256],
                            lhs, wp[t][:, u, kc, 512:O3],
                            start=(kc == 0), stop=(kc == 1),
                        )
            # sig = [rg | ig] per mc; 1/32 folded into the ACT scale
            sig = pgtmp.tile([128, MC, 512], f32, tag="sig")
            nc.scalar.activation(out=sig, in_=ri, func=AF.Sigmoid, scale=INV)
            t1 = pgtmp.tile([128, MC, H], f32, tag="t1")
            nc.vector.tensor_tensor(
                out=t1, in0=sig[:, :, 0:H], in1=nxh[:, :, 256:512], op=ALU.mult
            )
            t2 = pgtmp.tile([128, MC, H], f32, tag="t2")
            nc.vector.tensor_tensor(
                out=t2, in0=t1, in1=nxh[:, :, 0:256], op=ALU.add
            )
            stage2[u] = (sig, t2)

        def emit_back(u):
            sig, t2 = stage2.pop(u)
            # t2 still carries x32; fold 1/32 into the tanh scale
            ng = pgtmp.tile([128, MC, H], f32, tag="ng")
            nc.scalar.activation(out=ng, in_=t2, func=AF.Tanh, scale=INV)
            # last unit's d/e run on DVE (faster per op) to shorten the
            # post-matmul drain chain; GPSIMD otherwise
            deng = nc.vector if u == UC - 1 else nc.gpsimd
            d = pgtmp.tile([128, MC, H], f32, tag="d")
            deng.tensor_tensor(
                out=d, in0=hbh_sb[:, u, :, :], in1=ng, op=ALU.subtract
            )
            e = pgtmp.tile([128, MC, H], f32, tag="e")
            deng.tensor_tensor(
                out=e, in0=sig[:, :, 256:512], in1=d, op=ALU.mult
            )
            nc.vector.tensor_tensor(
                out=out_sb[u // 2][:, u % 2, :, :], in0=ng, in1=e, op=ALU.add
            )
            if u % 2 == 1:
                nc.sync.dma_start(
                    out=hy[:, u - 1 : u + 1, :, :],
                    in_=out_sb[u // 2][:, :, :, :],
                )

        with tc.tile_pool(name="pg", bufs=2, space="PSUM") as pg:
            for u in range(UC):
                emit_front(u, pg)
                if u >= 1:
                    emit_back(u - 1)
            emit_back(UC - 1)

    nc.compile()
    return nc


def _prep_inputs(x, hidden, pool_x, pool_h, sw_x, sw_h):
    """Host-side (free) slicing / transposition / casting per core."""
    # pool[s, o, d] -> lhsT layout [(s,oc), ob, kc, dm], o = ob*16+oc, d = kc*128+dm
    def prep_pool(p):
        pt = (p * WSCALE).reshape(S, NOB, 16, KC, 128)  # [s, ob, oc, kc, dm]
        pt = pt.transpose(0, 2, 1, 3, 4)                # [s, oc, ob, kc, dm]
        return np.ascontiguousarray(pt.reshape(128, NOB, KC, 128).astype(E3M4))

    poolx_h = prep_pool(pool_x)
    poolh_h = prep_pool(pool_h)

    in_maps = []
    for c in range(NCORES):
        us = slice(c * UC, (c + 1) * UC)

        def sw_block(sw_c):  # [UC, S] -> [(s,oc), (u,oc')] block-diagonal
            blk = np.zeros((S, 16, UC, 16), dtype=np.float32)
            for oc in range(16):
                blk[:, oc, :, oc] = sw_c.T
            return np.ascontiguousarray(blk.reshape(128, 128).astype(BF16))

        xc = x[:, us, :]       # [B, UC, DIN]
        hc = hidden[:, us, :]
        # [128 (d%128), UC, KC (d//128), B]
        xt_h = np.ascontiguousarray(
            xc.transpose(1, 2, 0).reshape(UC, KC, 128, B).transpose(2, 0, 1, 3).astype(BF16)
        )
        ht_h = np.ascontiguousarray(
            hc.transpose(1, 2, 0).reshape(UC, KC, 128, B).transpose(2, 0, 1, 3).astype(BF16)
        )
        # [128 (b%128), UC, MC (b//128), H]
        hbh_h = np.ascontiguousarray(
            hc.reshape(MC, 128, UC, H).transpose(1, 2, 0, 3).astype(BF16)
        )
        in_maps.append(
            {
                "poolx": poolx_h,
                "poolh": poolh_h,
                "swx": sw_block(sw_x[us]),
                "swh": sw_block(sw_h[us]),
                "xt": xt_h,
                "ht": ht_h,
                "hbh": hbh_h,
            }
        )
    return in_maps


_CACHED_NC = None


def _get_nc():
    global _CACHED_NC
    if _CACHED_NC is None:
        _CACHED_NC = _build_program()
    return _CACHED_NC


def kernel(x, hidden, pool_x, pool_h, sw_x, sw_h, _trace=False, _results_holder=None):
    from concourse.bass_utils import run_bass_kernel_spmd

    x = np.asarray(x)
    hidden = np.asarray(hidden)
    pool_x = np.asarray(pool_x)
    pool_h = np.asarray(pool_h)
    sw_x = np.asarray(sw_x)
    sw_h = np.asarray(sw_h)

    nc = _get_nc()
    in_maps = _prep_inputs(x, hidden, pool_x, pool_h, sw_x, sw_h)
    res = run_bass_kernel_spmd(
        nc, in_maps, core_ids=list(range(NCORES)), trace=_trace
    )
    if _results_holder is not None:
        _results_holder.append(res)

    out = np.empty((B, U, H), dtype=np.float32)
    for c in range(NCORES):
        hy_c = np.asarray(res.results[c]["hy"]).astype(np.float32)  # [128, UC, MC, H]
        # out[b, u, h] with b = mc*128 + bp
        out[:, c * UC : (c + 1) * UC, :] = hy_c.transpose(2, 0, 1, 3).reshape(B, UC, H)
    return out
